# revision 49
# baseline (speedup 1.0000x reference)
"""Trainium2 Bass kernel for nn_Attention_7679401525457.

score_i = relu(Linear(tanh(concat(h_i, z)))); alphas = softmax(scores);
attention = sum_i alphas_i * h_i.

Data-parallel over 8 NeuronCores: batch dim (32) sharded 4-per-core; the
tiny W/b replicated. Each core streams its 16 MiB encoder slice from HBM
exactly once in 512 KiB s-tiles on a dedicated DMA queue (SP HWDGE);
consts and outputs ride the Activation HWDGE queue so they never block
the stream. Per tile: ACT tanh (bf16 out) -> DVE fused multiply+reduce
in bf16 (4x DVE mode, f32 accumulate) -> relu+bias on DVE -> exp on ACT
-> PE accumulates the alpha-weighted sum AND the softmax denominator in
PSUM. Softmax normalization folds into the final PSUM->SBUF copies
(alphas stay unnormalized: relu keeps scores in [0, ~3], exp can't
overflow).
"""

import numpy as np

import concourse.bass as bass
import concourse.bacc as bacc
import concourse.mybir as mybir
import concourse.tile as tile
from concourse.bass_utils import run_bass_kernel_spmd

B, S, D = 32, 1024, 1024
NCORES = 8
BPC = B // NCORES  # batches per core
NT = S // 128  # s-tiles per batch (one 512 KiB DMA each)
F32 = mybir.dt.float32
F32R = mybir.dt.float32r
BF16 = mybir.dt.bfloat16
AF = mybir.ActivationFunctionType
ALU = mybir.AluOpType

# float32r: same bits as fp32, PE matmul runs 4x faster (TF32-like
# reduced mantissa in the array). Toggle if precision requires full fp32.
USE_F32R = True
# Offload the multiply of some tiles' score dot-products to the Pool
# (gpsimd) engine. The cost model likes it; on real HW gpsimd ops are
# far slower than modeled (measured: 99us vs 45us per iteration), so
# this stays empty.
POOL_PIECES = ()
# Software-pipeline depths: exp+matmul emitted EXP_SKEW tiles behind the
# score; score (accum/relu) emitted SCORE_DEFER pieces behind the
# front-end. Both keep the in-order sequencers from blocking on
# cross-engine dependencies.
EXP_SKEW = 2
SCORE_DEFER = 1
# Split each batch's last s-tile into two half-D DMAs to shorten the
# dependency chain behind the final transfer.
HALVES = True
# Pair adjacent full tiles into one 1 MiB DMA (halves HWDGE gen work;
# compute still per-tile via region-level deps).
DMA_PAIR = False
# Group G adjacent full tiles into one DMA (G*512 KiB). With
# LAYOUT="ptd" the per-partition segment is G*4 KiB contiguous.
DMA_GROUP = 1
# Split each full tile's DMA into two 256 KiB d-half transfers
# (finer DMA granularity; compute still per-tile).
DMA_SPLIT2 = False
# Split every batch's tail copies across DVE+ACT (not just the last
# batch): balances DVE (the tightest engine) against ACT slack.
TAIL_SPLIT = False
# tanh output double-buffer depth.
TT_BUFS = 4
# s-row -> (partition, tile) mapping. "tpd": row = t*128+p (4 KiB
# contiguous per partition per tile). "ptd": row = p*NT+t (adjacent
# tiles contiguous per partition, so paired DMAs get 8 KiB segments).
# Softmax/weighted-sum are permutation-invariant over s, so both work.
LAYOUT = "tpd"
# Diagnostic pipeline truncation: "full" | "dma" | "tanh" | "dot".
# Non-full stages produce WRONG output (dummy orow) - bench only.
STAGE = "full"
# Diagnostic: DMA only half of each piece's bytes (WRONG output).
DMA_HALF = False
# Stripe the enc stream across both HWDGE queues (SP + Activation).
DMA_Q2 = False

_CACHE = {}


def _build(loop=None):
    import contextlib

    encdt = F32R if USE_F32R else F32
    nc = bacc.Bacc("TRN2", target_bir_lowering=False, debug=False)

    enc = nc.dram_tensor("enc", [BPC, S, D], F32, kind="ExternalInput")
    # zt[p, b*8+c] = z[b, p*8+c]   (z = decoder_hidden[-1] core slice)
    zt = nc.dram_tensor("zt", [128, BPC * 8], F32, kind="ExternalInput")
    w1rep = nc.dram_tensor("w1rep", [128, D], BF16, kind="ExternalInput")
    # w2t[p, c] = W2[p*8+c]
    w2t = nc.dram_tensor("w2t", [128, 8], F32, kind="ExternalInput")
    # bb128 = b[0]/128 replicated, so a ones-matmul partition-sum adds b[0]
    bb128 = nc.dram_tensor("bb128", [128, 1], F32, kind="ExternalInput")
    att = nc.dram_tensor("att", [BPC, D], F32, kind="ExternalOutput")

    with tile.TileContext(nc) as tc:
        with (
            tc.tile_pool(name="const", bufs=1) as cpool,
            tc.tile_pool(name="encp", bufs=BPC) as encp,
            tc.tile_pool(name="ttp", bufs=TT_BUFS) as ttp,
            tc.tile_pool(name="junkp", bufs=2) as junkp,
            tc.tile_pool(name="pjunkp", bufs=2) as pjunkp,
            tc.tile_pool(name="smallp", bufs=4) as smallp,
            tc.tile_pool(name="orowp", bufs=2) as orowp,
            tc.tile_pool(name="pscb", bufs=1, space="PSUM") as pscb,
            tc.tile_pool(name="psp", bufs=2, space="PSUM") as psp,
            tc.tile_pool(name="aptp", bufs=2, space="PSUM") as aptp,
        ):
            # ---- constants (Activation HWDGE queue, off the enc stream) ----
            w1t = cpool.tile([128, D], BF16)
            nc.scalar.dma_start(w1t[:], w1rep.ap())
            ztt = cpool.tile([128, BPC * 8], F32)
            nc.scalar.dma_start(ztt[:], zt.ap())
            w2tt = cpool.tile([128, 8], F32)
            nc.scalar.dma_start(w2tt[:], w2t.ap())
            bbt = cpool.tile([128, 1], F32)
            nc.scalar.dma_start(bbt[:], bb128.ap())
            ones128 = cpool.tile([128, 1], F32)
            nc.vector.memset(ones128[:], 1.0)
            ones_sq = cpool.tile([128, 128], F32)
            nc.vector.memset(ones_sq[:], 1.0)

            # ---- prepass: cb[:, b] = tanh(z_b) @ W2 + b0, on all partitions
            tz = cpool.tile([128, BPC * 8], F32)
            nc.scalar.activation(tz[:], ztt[:], AF.Tanh)
            czp = cpool.tile([128, BPC], F32)
            zjunk = cpool.tile([128, 8], F32)
            for bi in range(BPC):
                nc.vector.scalar_tensor_tensor(
                    out=zjunk[:],
                    in0=tz[:, bi * 8 : (bi + 1) * 8],
                    scalar=1.0,
                    in1=w2tt[:],
                    op0=ALU.mult,
                    op1=ALU.mult,
                    accum_out=czp[:, bi : bi + 1],
                )
            czp2 = cpool.tile([128, BPC], F32)
            nc.vector.tensor_scalar_add(czp2[:], czp[:], bbt[:, 0:1])
            cb_ps = pscb.tile([128, BPC], F32)
            nc.tensor.matmul(cb_ps[:], ones_sq[:], czp2[:], start=True, stop=True)
            cb = cpool.tile([128, BPC], F32)
            nc.scalar.copy(cb[:], cb_ps[:])

            # ---- per-batch pipeline, streaming in 512 KiB s-tiles ----
            # Per tile: the fused multiply+reduce alternates DVE (even t)
            # and Pool (odd t) so neither engine saturates. Each batch's
            # normalization tail (recip + scaled PSUM copies + out-DMA) is
            # deferred one batch so it never stalls the streaming engines.
            # The last s-tile of each batch is split into two half-D pieces
            # to shorten the dependency chain behind the final DMA.
            ones_mm = ones128[:].bitcast(encdt) if USE_F32R else ones128[:]
            H = D // 2

            def emit_tail(bi, ps, last=False):
                ap0, ap1, apt = ps
                recip = smallp.tile([1, 1], F32, tag="recip")
                nc.vector.reciprocal(recip[:], apt[:])
                orow = orowp.tile([1, D], F32, tag="orow")
                nc.vector.tensor_scalar(
                    out=orow[:, 0:H],
                    in0=ap0[:],
                    scalar1=recip[0:1, 0:1],
                    scalar2=None,
                    op0=ALU.mult,
                )
                if last or TAIL_SPLIT:
                    # split across DVE+ACT so the exposed tail is short
                    nc.scalar.activation(
                        orow[:, H:D], ap1[:], AF.Copy, scale=recip[0:1, 0:1]
                    )
                else:
                    # hidden tail: keep it off the tightly-loaded ACT
                    nc.vector.tensor_scalar(
                        out=orow[:, H:D],
                        in0=ap1[:],
                        scalar1=recip[0:1, 0:1],
                        scalar2=None,
                        op0=ALU.mult,
                    )
                nc.scalar.dma_start(att.ap()[bi : bi + 1, :], orow[:])

            # exp+matmuls are emitted SKEW tiles behind the front-end
            # (DMA/tanh/dot/relu) so the in-order ACT sequencer never
            # blocks on the cross-engine score chain before an exp.
            SKEW = EXP_SKEW

            def emit_back(st):
                bi, t, sin, encT, al, ap0, ap1, apt = st
                nc.scalar.activation(al[:, t : t + 1], sin, AF.Exp)
                nc.tensor.matmul(
                    ap0[:],
                    al[:, t : t + 1],
                    encT[:, t * D : t * D + H],
                    start=(t == 0),
                    stop=(t == NT - 1),
                )
                nc.tensor.matmul(
                    ap1[:],
                    al[:, t : t + 1],
                    encT[:, t * D + H : (t + 1) * D],
                    start=(t == 0),
                    stop=(t == NT - 1),
                )
                # denominator mm in plain f32: fp32r forbids free-size-1
                nc.tensor.matmul(
                    apt[:],
                    al[:, t : t + 1].bitcast(F32) if USE_F32R else al[:, t : t + 1],
                    ones128[:],
                    start=(t == 0),
                    stop=(t == NT - 1),
                )

            lctx = tc.For_i(0, loop) if loop is not None else contextlib.nullcontext()
            with lctx:
              prev = None
              pending = []
              score_q = []
              for bi in range(BPC):
                encT = encp.tile([128, NT * D], encdt, tag="enc")
                if LAYOUT == "ptd":
                    src = enc.ap()[bi].rearrange("(p t) d -> p t d", p=128)
                else:
                    src = enc.ap()[bi].rearrange("(t p) d -> p t d", p=128)
                if USE_F32R:
                    src = src.bitcast(F32R)
                sc = smallp.tile([128, NT + 1], F32, tag="sc")
                sr = smallp.tile([128, NT], F32, tag="sr")
                al = smallp.tile([128, NT], encdt, tag="al")
                ap0 = psp.tile([1, H], F32, tag="ap0")
                ap1 = psp.tile([1, H], F32, tag="ap1")
                apt = aptp.tile([1, 1], F32, tag="apt")

                # (t, d0, d1, score_col) pieces; last tile split in half
                if HALVES:
                    pieces = [(t, 0, D, t) for t in range(NT - 1)]
                    pieces.append((NT - 1, 0, H, NT - 1))
                    pieces.append((NT - 1, H, D, NT))
                else:
                    pieces = [(t, 0, D, t) for t in range(NT)]

                def emit_score(job):
                    # deferred one piece so the in-order DVE never waits
                    # on the slower Pool multiply
                    (pl, n_, scol_, prod_, w_, sc_, sr_, al_, bi_, enc_,
                     a0_, a1_, at_) = job
                    if pl:  # row-sum of the Pool-made product (4x bf16)
                        junk = junkp.tile([128, D], BF16, tag="junk")
                        nc.vector.tensor_scalar(
                            out=junk[:, 0:n_],
                            in0=prod_[:, 0:n_],
                            scalar1=1.0,
                            scalar2=0.0,
                            op0=ALU.mult,
                            op1=ALU.add,
                            accum_out=sc_[:, scol_ : scol_ + 1],
                        )
                    if HALVES and scol_ == NT - 1:
                        return  # first half: second half finishes the tile
                    if scol_ == NT:  # combine the two half-sums
                        hs = smallp.tile([128, 1], F32, tag="hs")
                        nc.vector.tensor_scalar(
                            out=hs[:],
                            in0=sc_[:, NT - 1 : NT],
                            scalar1=sc_[:, NT : NT + 1],
                            scalar2=None,
                            op0=ALU.add,
                        )
                        sin = hs[:]
                        t = NT - 1
                    else:
                        sin = sc_[:, scol_ : scol_ + 1]
                        t = scol_
                    # relu(score + cb) in one DVE op; exp deferred by SKEW.
                    nc.vector.tensor_scalar(
                        out=sr_[:, t : t + 1],
                        in0=sin,
                        scalar1=cb[:, bi_ : bi_ + 1],
                        scalar2=0.0,
                        op0=ALU.add,
                        op1=ALU.max,
                    )
                    pending.append((bi_, t, sr_[:, t : t + 1], enc_, al_,
                                    a0_, a1_, at_))
                    if len(pending) > SKEW:
                        emit_back(pending.pop(0))

                nfull = sum(1 for p in pieces if p[2] - p[1] == D)
                for pi, (ti, d0, d1, scol) in enumerate(pieces):
                    cols = slice(ti * D + d0, ti * D + d1)
                    full = d1 - d0 == D
                    G = 2 if DMA_PAIR else DMA_GROUP
                    if G > 1 and full and ti % G == 0:
                        g = min(G, nfull - ti)
                        nc.sync.dma_start(
                            encT[:, ti * D : (ti + g) * D].rearrange(
                                "p (t d) -> p t d", t=g
                            ),
                            src[:, ti : ti + g, :],
                        )
                    elif G > 1 and full and ti % G != 0:
                        pass  # covered by the group DMA
                    else:
                        deng = nc.scalar if (DMA_Q2 and pi % 2) else nc.sync
                        if DMA_HALF:
                            h2 = (d1 - d0) // 2
                            deng.dma_start(
                                encT[:, cols.start : cols.start + h2],
                                src[:, ti, d0 : d0 + h2],
                            )
                        elif DMA_SPLIT2 and full:
                            deng.dma_start(
                                encT[:, cols.start : cols.start + H],
                                src[:, ti, d0 : d0 + H],
                            )
                            deng.dma_start(
                                encT[:, cols.start + H : cols.stop],
                                src[:, ti, d0 + H : d1],
                            )
                        else:
                            deng.dma_start(encT[:, cols], src[:, ti, d0:d1])
                    n = d1 - d0
                    if n == D:
                        tt = ttp.tile([128, D], BF16, tag="tt")
                    else:
                        tt = ttp.tile([128, H], BF16, tag="tth")
                    if STAGE == "dma":
                        continue
                    tin = encT[:, cols]
                    if USE_F32R:
                        tin = tin.bitcast(F32)
                    nc.scalar.activation(tt[:, 0:n], tin, AF.Tanh)
                    if STAGE == "tanh":
                        continue
                    # multiply+row-sum, bf16 in/out, f32 accumulate.
                    # Middle tiles: Pool does the multiply (tensor_tensor,
                    # the only DVE-style op legal on Pool), DVE row-sums
                    # via tensor_scalar+accum (4x bf16 mode, ~420ns).
                    # Other tiles: fused scalar_tensor_tensor on DVE
                    # (tensor_tensor_reduce crashes the exec unit on this
                    # runtime; scalar_tensor_tensor accum works).
                    wslice = w1t[:, d0:d1] if n != D else w1t[:]
                    pl = pi in POOL_PIECES
                    prod = None
                    if pl:
                        prod = pjunkp.tile([128, D], BF16, tag="prod")
                        nc.gpsimd.tensor_tensor(
                            out=prod[:, 0:n],
                            in0=tt[:, 0:n],
                            in1=wslice,
                            op=ALU.mult,
                        )
                    else:
                        junk = junkp.tile([128, D], BF16, tag="junk")
                        nc.vector.scalar_tensor_tensor(
                            out=junk[:, 0:n],
                            in0=tt[:, 0:n],
                            scalar=1.0,
                            in1=wslice,
                            op0=ALU.mult,
                            op1=ALU.mult,
                            accum_out=sc[:, scol : scol + 1],
                        )
                    if STAGE == "dot":
                        continue
                    score_q.append((pl, n, scol, prod, wslice, sc, sr, al,
                                    bi, encT, ap0, ap1, apt))
                    while len(score_q) > SCORE_DEFER:
                        emit_score(score_q.pop(0))

                if STAGE != "full":
                    orow = orowp.tile([1, D], F32, tag="orow")
                    nc.vector.memset(orow[:], float(bi))
                    nc.scalar.dma_start(att.ap()[bi : bi + 1, :], orow[:])
                    continue
                if prev is not None:
                    emit_tail(prev[0], prev[1])
                prev = (bi, (ap0, ap1, apt))
              if STAGE == "full":
                for job in score_q:
                    emit_score(job)
                score_q = []
                for st in pending:
                    emit_back(st)
                pending = []
                emit_tail(prev[0], prev[1], last=True)

    nc.compile()
    return nc


def _get_nc():
    if "nc" not in _CACHE:
        _CACHE["nc"] = _build()
    return _CACHE["nc"]


def _make_in_maps(encoder_outputs, decoder_hidden, W, b):
    import ml_dtypes

    enc = np.ascontiguousarray(np.asarray(encoder_outputs, dtype=np.float32))
    z = np.asarray(decoder_hidden, dtype=np.float32)[-1]  # [B, D]
    W = np.asarray(W, dtype=np.float32)
    b = np.asarray(b, dtype=np.float32)

    W1 = W[:D, 0]
    W2 = W[D:, 0]
    w1rep = np.ascontiguousarray(
        np.broadcast_to(W1[None, :], (128, D)).astype(ml_dtypes.bfloat16)
    )
    w2t = np.ascontiguousarray(W2.reshape(128, 8))
    bb128 = np.full((128, 1), float(b[0]) / 128.0, dtype=np.float32)

    in_maps = []
    for c in range(NCORES):
        zi = z[c * BPC : (c + 1) * BPC]  # [BPC, D]
        ztc = np.ascontiguousarray(
            zi.reshape(BPC, 128, 8).transpose(1, 0, 2).reshape(128, BPC * 8)
        )
        in_maps.append(
            {
                "enc": np.ascontiguousarray(enc[c * BPC : (c + 1) * BPC]),
                "zt": ztc,
                "w1rep": w1rep,
                "w2t": w2t,
                "bb128": bb128,
            }
        )
    return in_maps


def kernel(encoder_outputs, decoder_hidden, W, b, **_):
    in_maps = _make_in_maps(encoder_outputs, decoder_hidden, W, b)
    nc = _get_nc()
    res = run_bass_kernel_spmd(nc, in_maps, list(range(NCORES)))
    out = np.concatenate([res.results[c]["att"] for c in range(NCORES)], axis=0)
    return out.astype(np.float32)
